# revision 9
# baseline (speedup 1.0000x reference)
"""Trainium2 Bass kernel for nn_Capsule: capsule routing head.

Math: the einsum 'nco,pbo->bno' factorizes as xp[b,n,o] = W[n,o] * X[b,o]
with W = caps_weights.sum(c) (64x128) and X = x.sum(p) (256x128), so the
kernel is a memory-bound reduction of x (151 MB) followed by a tiny
per-batch routing loop (matmuls of size <= 128x64x128).

Sharding: data-parallel over batch (dim 1 of x), 32 batch elements per
core; caps_weights replicated; no cross-core communication.

Per-core pipeline (v2):
  - x streams over both HWDGE rings (scalar + sync) as half-slab DMAs
    (8 KB per partition line); the last two slabs are batch-quartered so
    the final dependency unit is 8 batch rows.  The scalar ring leads
    (it exits the entry barrier first); w rides behind the first slab.
  - Reduction via fp32r matmuls with one-hot-column stationaries built
    on-chip (no cst input): slabs 0-5 as 2-slab pairs (256-wide moving),
    slabs 6/7/8 as single-slab 128-wide matmuls whose reads stay inside
    one DMA's written range, keeping the tail deps fine-grained.  All
    192 matmuls accumulate into ONE psum tile; a single strided reduce
    combines the two 128-wide sub-columns.
  - Routing in b-on-partitions layout; transposes via DVE 32x32 block
    transposes (no PE/psum round-trips); sum-of-squares fused into one
    tensor_tensor_reduce; squash scale = Exp(0.5*Ln(q) + (-Ln(1+q)))
    using the per-partition activation bias port, with -Ln(1+q) taken
    as Ln(1/(1+q)) off the DVE reciprocal.
"""

import numpy as np

# ---- problem constants (hardcoded per contract) ----
P_TOT = 1152
BATCH = 256
O = 128
N_CAPS = 64
CAPS_DIM = 16
ITERATIONS = 3
N_CORES = 8
B_LOC = BATCH // N_CORES          # 32 batch elements per core
PT = P_TOT // 128                 # 9 p-tiles (slabs)
SLAB = B_LOC * O                  # 4096 f32 per partition-slab row

_cache = {}


def _pin_act_table():
    """Force every ACT function onto the one table containing
    Exp+Ln+Square+Copy, so the kernel needs a single ACT_TABLE_LOAD."""
    import functools
    import concourse.hw_specs as hw_specs
    import concourse.bacc as bacc_mod

    if getattr(hw_specs.get_activation_tables, "_capsule_pinned", False):
        return
    orig = hw_specs.get_activation_tables

    @functools.cache
    def pinned(module_arch):
        tabs = orig(module_arch)
        keep = None
        for name, fns in tabs.items():
            names = {f.name for f in fns}
            if {"Exp", "Ln", "Square", "Copy", "Identity"} <= names:
                keep = name
                break
        if keep is None:
            return tabs
        return {n: (fns if n == keep else type(fns)()) for n, fns in tabs.items()}

    pinned._capsule_pinned = True
    hw_specs.get_activation_tables = pinned
    bacc_mod.get_activation_tables = pinned


def _build():
    _pin_act_table()
    import concourse.bacc as bacc
    import concourse.tile as tile
    import concourse.mybir as mybir

    f32 = mybir.dt.float32
    f32r = mybir.dt.float32r
    AX = mybir.AxisListType
    AF = mybir.ActivationFunctionType
    OP = mybir.AluOpType

    nc = bacc.Bacc(None, target_bir_lowering=False)

    # x declared f32r: same bytes as fp32, lets plain HWDGE DMAs feed the
    # fast fp32r matmul path with no cast.
    x_in = nc.dram_tensor("x", [P_TOT, B_LOC, O], f32r, kind="ExternalInput")
    w_in = nc.dram_tensor("caps_weights", [N_CAPS, CAPS_DIM, O], f32,
                          kind="ExternalInput")
    out_d = nc.dram_tensor("out", [B_LOC, O], f32, kind="ExternalOutput")

    xv = x_in.rearrange("(t p) b o -> t p b o", p=128)   # (9, 128, 32, 128)

    with tile.TileContext(nc) as tc:
        with (
            tc.tile_pool(name="xin", bufs=6) as xpool,
            tc.tile_pool(name="wrk", bufs=1) as wrk,
            tc.tile_pool(name="small", bufs=1) as small,
            tc.tile_pool(name="ps", bufs=1, space="PSUM") as ps,
        ):
            # slab storage: 3 pair tiles (slabs 0-5), 3 single tiles (6,7,8)
            xg = [xpool.tile([128, 2 * SLAB], f32r, tag="xg", name=f"xg{p}")
                  for p in range(3)]
            xs = {t: xpool.tile([128, SLAB], f32r, tag="xg", name=f"xs{t}")
                  for t in (6, 7, 8)}

            def slab_dst(t):
                if t < 6:
                    p, s = divmod(t, 2)
                    return xg[p][:, s * SLAB:(s + 1) * SLAB]
                return xs[t][:]

            def dma_half(eng, t, h):
                b0 = h * (B_LOC // 2)
                b1 = b0 + B_LOC // 2
                dst = slab_dst(t).rearrange("p (b o) -> p b o", b=B_LOC)
                eng.dma_start(dst[:, b0:b1, :], xv[t][:, b0:b1, :])

            def dma_quarter(eng, t, q):
                b0 = q * (B_LOC // 4)
                b1 = b0 + B_LOC // 4
                dst = slab_dst(t).rearrange("p (b o) -> p b o", b=B_LOC)
                eng.dma_start(dst[:, b0:b1, :], xv[t][:, b0:b1, :])

            # ---- DMA issue: scalar ring leads with x; w behind slab 0 ----
            w_sb = wrk.tile([N_CAPS, CAPS_DIM * O], f32)
            dma_half(nc.scalar, 0, 0)
            dma_half(nc.sync, 1, 0)
            dma_half(nc.scalar, 0, 1)
            dma_half(nc.sync, 1, 1)
            nc.scalar.dma_start(w_sb[:], w_in.rearrange("n c o -> n (c o)"))
            for t in (2, 4, 6):
                for h in (0, 1):
                    dma_half(nc.scalar, t, h)
                    dma_half(nc.sync, t + 1, h)
            # tail: slab 7 quartered on sync, slab 8 quartered across both
            for q in range(4):
                dma_quarter(nc.sync, 7, q)
            dma_quarter(nc.scalar, 8, 0)
            dma_quarter(nc.scalar, 8, 1)
            dma_quarter(nc.sync, 8, 2)
            dma_quarter(nc.sync, 8, 3)

            # one-hot stationary source built on-chip: (128, 63) with ones
            # in column 31, so zpat[:, 31-b : 63-b] is one-hot-column-b.
            # memset requires a plain-f32 view; matmuls bitcast to f32r.
            zpat_f = small.tile([128, 2 * B_LOC - 1], f32)
            nc.gpsimd.memset(zpat_f[:], 0.0)
            nc.gpsimd.memset(zpat_f[:, B_LOC - 1:B_LOC], 1.0)

            def zpat(b):
                return zpat_f[:, B_LOC - 1 - b: 2 * B_LOC - 1 - b].bitcast(f32r)

            # ---- capsule weight prep (overlaps the x stream) ----
            # fold the 16 caps_dim rows in place on w_sb to save SBUF
            nc.vector.tensor_tensor(w_sb[:, :8 * O], w_sb[:, :8 * O],
                                    w_sb[:, 8 * O:], OP.add)
            nc.vector.tensor_tensor(w_sb[:, :4 * O], w_sb[:, :4 * O],
                                    w_sb[:, 4 * O:8 * O], OP.add)
            nc.vector.tensor_tensor(w_sb[:, :2 * O], w_sb[:, :2 * O],
                                    w_sb[:, 2 * O:4 * O], OP.add)
            w_no = wrk.tile([N_CAPS, O], f32)          # W[n,o]
            nc.vector.tensor_tensor(w_no[:], w_sb[:, :O], w_sb[:, O:2 * O], OP.add)

            # W^T[o,n] via PE transpose (DVE stream-transpose is not
            # supported on this runtime path)
            from concourse.masks import make_identity
            ident = small.tile([128, 128], f32)
            make_identity(nc, ident[:])
            ps_wt = ps.tile([O, N_CAPS], f32, tag="ps_wt")
            nc.tensor.transpose(ps_wt[:], w_no[:], ident[:N_CAPS, :N_CAPS])
            wt_on = wrk.tile([O, N_CAPS], f32)
            nc.vector.tensor_copy(wt_on[:], ps_wt[:])

            # S0[b,o] = (1/64) sum_n W[n,o] for every b (uniform coeffs0);
            # also warms the PE early.
            unif = small.tile([N_CAPS, B_LOC], f32)
            nc.gpsimd.memset(unif[:], 1.0 / N_CAPS)
            ps_s0 = ps.tile([B_LOC, O], f32, tag="ps_s0")
            nc.tensor.matmul(ps_s0[:], unif[:], w_no[:], start=True, stop=True)

            # ---- reduction: X[b,o] = sum_p x[p,b,o] ----
            # one accumulation group of 192 matmuls into ps_x[32, 256]:
            #   pairs (0,1),(2,3),(4,5): 256-wide moving (b, s, o) strided
            #   singles 6,7,8: 128-wide moving into sub-column 0
            ps_x = ps.tile([B_LOC, 2 * O], f32, tag="ps_x")
            for p in range(3):
                mvv = xg[p][:].rearrange("p (s b o) -> p b s o",
                                         b=B_LOC, s=2)
                for b in range(B_LOC):
                    nc.tensor.matmul(
                        ps_x[:], zpat(b),
                        mvv[:, b, :, :],
                        start=(p == 0 and b == 0),
                        stop=False,
                        skip_group_check=True)
            for t in (6, 7, 8):
                for b in range(B_LOC):
                    nc.tensor.matmul(
                        ps_x[:, :O], zpat(b),
                        xs[t][:, b * O:(b + 1) * O],
                        start=False,
                        stop=(t == 8 and b == B_LOC - 1),
                        skip_group_check=True)

            x32 = wrk.tile([B_LOC, O], f32)             # X[b,o]
            nc.vector.tensor_reduce(
                x32[:], ps_x[:].rearrange("p (s o) -> p o s", s=2),
                AX.X, OP.add)

            # ---- routing (b on partitions) ----
            u = wrk.tile([B_LOC, O], f32)
            ue = wrk.tile([B_LOC, O], f32)
            sq = wrk.tile([B_LOC, O], f32)
            ux = wrk.tile([B_LOC, O], f32)
            tb = wrk.tile([B_LOC, O], f32)
            tT = wrk.tile([O, B_LOC], f32)
            exT = wrk.tile([N_CAPS, B_LOC], f32)
            logits = wrk.tile([B_LOC, N_CAPS], f32)
            lg2 = wrk.tile([B_LOC, N_CAPS], f32)
            ex = wrk.tile([B_LOC, N_CAPS], f32)
            nsq = wrk.tile([B_LOC, 1], f32)
            den = wrk.tile([B_LOC, 1], f32)
            rden = wrk.tile([B_LOC, 1], f32)
            lnq = wrk.tile([B_LOC, 1], f32)
            nls = wrk.tile([B_LOC, 1], f32)
            sct = wrk.tile([B_LOC, 1], f32)
            ssum = wrk.tile([B_LOC, 1], f32)
            rsum = wrk.tile([B_LOC, 1], f32)
            out_sb = wrk.tile([B_LOC, O], f32)

            def squash_scale():
                # sct = sqrt(q)/(1+q) = Exp(0.5*Ln(q) + Ln(1/(1+q)));
                # DVE recip overlaps the first ACT lookup.
                nc.vector.tensor_scalar_add(den[:], nsq[:], 1.0)
                nc.vector.reciprocal(rden[:], den[:])
                nc.scalar.activation(lnq[:], nsq[:], AF.Ln)
                nc.scalar.activation(nls[:], rden[:], AF.Ln)
                nc.scalar.activation(sct[:], lnq[:], AF.Exp,
                                     bias=nls[:], scale=0.5)


            for it in range(ITERATIONS):
                if it == 0:
                    nc.vector.tensor_tensor(u[:], x32[:], ps_s0[:], OP.mult)
                else:
                    ps_s = ps.tile([B_LOC, O], f32, tag="ps_s",
                                   name=f"ps_s{it}")
                    nc.tensor.matmul(ps_s[:], exT[:], w_no[:],
                                     start=True, stop=True)
                    nc.vector.tensor_tensor(ue[:], x32[:], ps_s[:], OP.mult)
                    nc.vector.tensor_scalar_mul(u[:], ue[:], rsum[:])
                nc.vector.tensor_tensor(sq[:], u[:], u[:], OP.mult)
                nc.vector.tensor_reduce(nsq[:], sq[:], AX.X, OP.add)
                if it < ITERATIONS - 1:
                    nc.vector.tensor_tensor(ux[:], u[:], x32[:], OP.mult)
                squash_scale()

                if it < ITERATIONS - 1:
                    # t = routed*X = sct*u*X ; delta[b,n] = sum_o t W^T
                    nc.vector.tensor_scalar_mul(tb[:], ux[:], sct[:])
                    ps_t = ps.tile([O, B_LOC], f32, tag="ps_t",
                                   name=f"ps_t{it}")
                    nc.tensor.transpose(ps_t[:], tb[:], ident[:B_LOC, :B_LOC])
                    nc.vector.tensor_copy(tT[:], ps_t[:])
                    ps_d = ps.tile([B_LOC, N_CAPS], f32, tag="ps_d",
                                   name=f"ps_d{it}")
                    nc.tensor.matmul(ps_d[:], tT[:], wt_on[:],
                                     start=True, stop=True)
                    # softmax over n (free axis); normalization deferred
                    if it == 0:
                        nc.scalar.activation(ex[:], ps_d[:], AF.Exp,
                                             accum_out=ssum[:])
                        nc.vector.tensor_copy(logits[:], ps_d[:])
                    else:
                        nc.vector.tensor_tensor(lg2[:], logits[:], ps_d[:],
                                                OP.add)
                        nc.scalar.activation(ex[:], lg2[:], AF.Exp,
                                             accum_out=ssum[:])
                    nc.vector.reciprocal(rsum[:], ssum[:])
                    ps_ct = ps.tile([N_CAPS, B_LOC], f32, tag="ps_ct",
                                    name=f"ps_ct{it}")
                    nc.tensor.transpose(ps_ct[:], ex[:], ident[:B_LOC, :B_LOC])
                    nc.vector.tensor_copy(exT[:], ps_ct[:])
                else:
                    nc.vector.tensor_scalar_mul(out_sb[:], u[:], sct[:])
                    nc.scalar.dma_start(out_d[:], out_sb[:])

    nc.compile()
    return nc


def run_with_results(x: np.ndarray, caps_weights: np.ndarray, **run_kwargs):
    """Run the SPMD kernel; returns (output (256,1,128), BassKernelResults)."""
    from concourse.bass_utils import run_bass_kernel_spmd

    if "nc" not in _cache:
        _cache["nc"] = _build()
    nc = _cache["nc"]

    x = np.ascontiguousarray(x, dtype=np.float32)
    caps_weights = np.ascontiguousarray(caps_weights, dtype=np.float32)

    in_maps = []
    for c in range(N_CORES):
        in_maps.append({
            "x": np.ascontiguousarray(x[:, c * B_LOC:(c + 1) * B_LOC, :]),
            "caps_weights": caps_weights,
        })
    res = run_bass_kernel_spmd(nc, in_maps, core_ids=list(range(N_CORES)),
                               **run_kwargs)
    out = np.concatenate([res.results[c]["out"] for c in range(N_CORES)], axis=0)
    return out.reshape(BATCH, 1, O), res


def kernel(x: np.ndarray, caps_weights: np.ndarray) -> np.ndarray:
    out, _ = run_with_results(x, caps_weights)
    return out
